# revision 81
# baseline (speedup 1.0000x reference)
"""GNN message passing (3x GraphConv+BN+ReLU, final GraphConv) on 8 trn2 cores.

Strategy (src-side partial aggregation + ReduceScatter):
  - Nodes are partitioned 6250/core; per dst core they are vector-packed into
    51 chunks of 128 slots so that every (src core, dst chunk) pair has at
    most 256 in-edges (2 tiles of 128).
  - Each core processes the edges whose SOURCE it owns: it gathers source
    rows from its local node-major fp16 table (int16 indices, one table),
    segment-sums them into per-destination-bin partial aggregates with
    one-hot matmuls in PSUM, and stages the fp16 feature-major partials into
    a [8*128, 6528] DRAM buffer ordered by destination core.
  - One ReduceScatter(add) per layer hands every core the summed aggregate
    for its own 6528 slots — ~57us vs ~251us for a feature AllGather.
  - Dense transforms, BatchNorm (stats corrected for the pad slots via a
    reserved always-pad column + tiny AllGather), ReLU, and the transpose
    back to the node-major fp16 gather table are all local.
  - The final GraphConv projects the partial aggregates to OUT=2 before the
    exchange, shrinking the last ReduceScatter to [8*128, 102] fp32 (~16us).
"""

import sys

import numpy as np

sys.path.insert(0, "/opt/trn_rl_repo")

import concourse.bass as bass  # noqa: E402
import concourse.mybir as mybir  # noqa: E402
import concourse.tile as tile  # noqa: E402
from concourse.vector_clock import ScopedClock  # noqa: E402
from concourse import library_config  # noqa: E402
from concourse.library_overlay import lower_extended_insts  # noqa: E402

N = 50000
E = 800000
D = 128
L = 3
OUT = 2
EPS = 1e-5
N_CORES = 8
CHUNKS = 51
P = 128
SLOTS = CHUNKS * P                  # 6528 slots per core
NODES_PER_CORE = N // N_CORES       # 6250
TPB = 2                             # tiles per (src core, dst bin)
BIN_CAP = TPB * P                   # 256
GBINS = N_CORES * CHUNKS            # 408 global bins
THIRDS = 3
TH_CH = [21, 21, 9]                 # chunks per third (last small so its
TH_OFF = [0, 21, 42]                # ReduceScatter is cheap and late)
TH_TILES = [c * TPB for c in TH_CH]           # tiles per (third, dst core)
TH_NIDX = [t * P for t in TH_TILES]           # gather idxs per call
TH_ICOLS = [n // 16 for n in TH_NIDX]         # idx cols per call
TILES = GBINS * TPB                 # 816 tiles per core per layer
IDX_COLS_TOTAL = N_CORES * sum(TH_ICOLS)      # 6528
N_PADS = SLOTS - NODES_PER_CORE     # 278 pad slots per core

# per-(third, dst core) gather-call offsets in tiles and idx columns
CALL_TILE_OFF = {}
CALL_ICOL_OFF = {}
_to = _io = 0
for _th in range(THIRDS):
    for _rd in range(N_CORES):
        CALL_TILE_OFF[(_th, _rd)] = _to
        CALL_ICOL_OFF[(_th, _rd)] = _io
        _to += TH_TILES[_th]
        _io += TH_ICOLS[_th]
assert _to == TILES and _io == IDX_COLS_TOTAL

F16 = mybir.dt.float16
F32 = mybir.dt.float32
I16 = mybir.dt.int16

# ---------------------------------------------------------------------------
# walrus in this container accepts at most ONE semaphore wait per instruction.
# Patch the Tile exit drain and add a post-pass splitting multi-wait insts.
# ---------------------------------------------------------------------------
_MAX_WAITS = 1


def _drain_and_barrier(self, tick_clock, wait_clock):
    nc = self.nc
    drain_inst = nc.sync.drain()
    wait_clock.add_sem_waits(
        drain_inst.ins, ScopedClock({None: tick_clock.global_clock})
    )
    si = drain_inst.ins.sync_info
    if si is not None and si.on_wait is not None and len(si.on_wait) > _MAX_WAITS:
        waits = list(si.on_wait)
        si.on_wait = waits[:_MAX_WAITS]
        rest = waits[_MAX_WAITS:]
        for i in range(0, len(rest), _MAX_WAITS):
            nop = nc.sync.nop(nofuse=True)
            nop.ins.sync_info = mybir.SyncInfo(
                on_wait=rest[i : i + _MAX_WAITS], on_update=[]
            )
    nc.all_engine_barrier()
    assert self.sems is not None
    popped = nc._tile_sem_poison_stack.pop()
    assert popped is self._sem_poison
    nc.clear_and_free_semaphores(list(self.sems.allocated().values()))
    nc.all_engine_barrier()


tile.TileContext._drain_and_barrier = _drain_and_barrier


def _split_multiwait(nc):
    n_split = 0
    for fn in nc.m.functions:
        for blk in fn.blocks:
            out = []
            for inst in blk.instructions:
                si = inst.sync_info
                if si is not None and si.on_wait and len(si.on_wait) > _MAX_WAITS:
                    waits = list(si.on_wait)
                    si.on_wait = waits[-_MAX_WAITS:]
                    rest = waits[:-_MAX_WAITS]
                    for i in range(0, len(rest), _MAX_WAITS):
                        n_split += 1
                        out.append(
                            mybir.InstNoOp(
                                name=f"{inst.name}-ws{i}",
                                engine=inst.engine,
                                ins=[],
                                outs=[],
                                bass_nofuse=True,
                                sync_info=mybir.SyncInfo(
                                    on_wait=rest[i : i + _MAX_WAITS], on_update=[]
                                ),
                                debug=inst.debug,
                            )
                        )
                out.append(inst)
            blk.instructions[:] = out
    return n_split


# ---------------------------------------------------------------------------
# Host-side graph partitioning
# ---------------------------------------------------------------------------
def _partition_nodes(src, dst):
    outdeg = np.bincount(src, minlength=N)
    order = np.argsort(-outdeg, kind="stable")
    core_of = np.empty(N, np.int32)
    core_of[order] = np.arange(N) % N_CORES

    d = np.zeros((N, N_CORES), np.int64)
    np.add.at(d, (dst, core_of[src]), 1)

    chunk_of = np.empty(N, np.int32)
    slot_of = np.empty(N, np.int32)
    slot_cap = np.full(CHUNKS, P, np.int64)
    slot_cap[CHUNKS - 1] = P - 1  # slot (50,127) reserved: always a pad
    for c in range(N_CORES):
        nodes = np.where(core_of == c)[0]
        dn = d[nodes]
        sub = np.argsort(-dn.sum(axis=1), kind="stable")
        fill_e = np.zeros((CHUNKS, N_CORES), np.int64)
        fill_s = np.zeros(CHUNKS, np.int64)
        for i in sub:
            v = dn[i]
            feas = (fill_s < slot_cap) & (fill_e + v <= BIN_CAP).all(axis=1)
            if not feas.any():
                raise RuntimeError("vector packing infeasible")
            newmax = (fill_e + v).max(axis=1)
            newmax[~feas] = 1 << 30
            k = int(np.argmin(newmax))
            n = nodes[i]
            chunk_of[n] = k
            slot_of[n] = fill_s[k]
            fill_s[k] += 1
            fill_e[k] += v
    return core_of, chunk_of, slot_of


def _preprocess(x, edge_index):
    x = np.asarray(x, np.float32)
    ei = np.asarray(edge_index)
    src = ei[0].astype(np.int64)
    dst = ei[1].astype(np.int64)
    core_of, chunk_of, slot_of = _partition_nodes(src, dst)

    scr = core_of[src]
    lrow = slot_of[src].astype(np.int64) * CHUNKS + chunk_of[src]
    gbin = core_of[dst].astype(np.int64) * CHUNKS + chunk_of[dst]
    dslot = slot_of[dst]

    gidx = np.zeros((N_CORES, TILES, P), np.int16)
    dstv = np.full((N_CORES, TILES, P), -1.0, np.float16)
    for r in range(N_CORES):
        m = scr == r
        b = gbin[m]
        order = np.argsort(b, kind="stable")
        b = b[order]
        lr = lrow[m][order]
        ds = dslot[m][order]
        counts = np.bincount(b, minlength=GBINS)
        assert counts.max() <= BIN_CAP
        starts = np.concatenate([[0], np.cumsum(counts)[:-1]])
        pos = np.arange(b.size) - starts[b]
        flat = b * BIN_CAP + pos
        gidx[r].reshape(-1)[flat] = lr.astype(np.int16)
        dstv[r].reshape(-1)[flat] = ds.astype(np.float16)

    # Reorder tiles to (third, dst core, chunk-in-third) so that each
    # gather call covers exactly the bins of one (third, dst core) and the
    # three ReduceScatters cover contiguous tile ranges.
    perm = np.array(
        [(rd * CHUNKS + TH_OFF[th] + cl) * TPB + t
         for th in range(THIRDS) for rd in range(N_CORES)
         for cl in range(TH_CH[th]) for t in range(TPB)],
        np.int64,
    )
    gidx = gidx[:, perm, :]
    dstv = dstv[:, perm, :]

    # dma_gather idx layout: index i of a call at [i % 16, i // 16],
    # replicated to 128 partitions; calls concatenated along columns
    idx_cores = np.empty((N_CORES, P, IDX_COLS_TOTAL), np.int16)
    for r in range(N_CORES):
        for th in range(THIRDS):
            for rd in range(N_CORES):
                t0 = CALL_TILE_OFF[(th, rd)]
                i0 = CALL_ICOL_OFF[(th, rd)]
                ic = TH_ICOLS[th]
                flat = gidx[r, t0 : t0 + TH_TILES[th]].reshape(-1)
                w = flat.reshape(ic, 16).T                     # [16, cols]
                full = np.broadcast_to(
                    w[None, :, :], (8, 16, ic)
                ).reshape(P, ic)
                idx_cores[r][:, i0 : i0 + ic] = full

    dstv_cores = np.ascontiguousarray(dstv.transpose(0, 2, 1))  # [8, 128, 816]

    col_of = chunk_of.astype(np.int64) * P + slot_of
    row_of = slot_of.astype(np.int64) * CHUNKS + chunk_of
    x_nm = np.zeros((N_CORES, SLOTS, D), np.float16)
    xT = np.zeros((N_CORES, D, SLOTS), np.float16)
    for r in range(N_CORES):
        m = core_of == r
        x_nm[r, row_of[m]] = x[m].astype(np.float16)
        xT[r][:, col_of[m]] = x[m].T.astype(np.float16)

    # layer-0 pre-gather: edge-ordered x rows in the gbuf SBUF layout
    # xg[p, g*4352 + t*128 + f] = x_nm[gidx[g*34+t, p], f]
    xg = np.empty((N_CORES, P, TILES * D), np.float16)
    for r in range(N_CORES):
        gat = x_nm[r][gidx[r].astype(np.int64)]      # [TILES, P, D]
        pad = dstv[r] < 0
        gat[pad] = 0
        xg[r] = gat.transpose(1, 0, 2).reshape(P, TILES * D)
    return core_of, chunk_of, slot_of, idx_cores, dstv_cores, x_nm, xT, xg


# ---------------------------------------------------------------------------
# Device program
# ---------------------------------------------------------------------------
def build_program():
    nc = bass.Bass(num_devices=N_CORES)

    p_xg = nc.declare_dram_parameter("xg", [P, TILES * D], F16, isOutput=False)
    p_xT = nc.declare_dram_parameter("xT_loc", [D, SLOTS], F16, isOutput=False)
    p_idx = nc.declare_dram_parameter("gidx", [P, IDX_COLS_TOTAL], I16, isOutput=False)
    p_dstv = nc.declare_dram_parameter("dstv", [P, TILES], F16, isOutput=False)
    p_wrel = nc.declare_dram_parameter("wrel", [L, D, D], F16, isOutput=False)
    p_wroot = nc.declare_dram_parameter("wroot", [L, D, D], F16, isOutput=False)
    p_w2 = nc.declare_dram_parameter("w2", [D, 2 * OUT], F16, isOutput=False)
    p_bR = nc.declare_dram_parameter("bR", [1, L * D], F16, isOutput=False)
    p_b2 = nc.declare_dram_parameter("b2", [1, OUT], F16, isOutput=False)
    p_gammaT = nc.declare_dram_parameter("gammaT", [D, L], F32, isOutput=False)
    p_betaT = nc.declare_dram_parameter("betaT", [D, L], F32, isOutput=False)
    p_iota = nc.declare_dram_parameter("iota16", [P, P], F16, isOutput=False)
    p_ident = nc.declare_dram_parameter("ident16", [P, P], F16, isOutput=False)
    p_out = nc.declare_dram_parameter("z4", [P, CHUNKS * OUT], F32, isOutput=True)

    rg = [list(range(N_CORES))]

    from contextlib import ExitStack

    with tile.TileContext(nc) as tc:
        with ExitStack() as _stack:
            _p = lambda *a, **k: _stack.enter_context(tc.tile_pool(*a, **k))  # noqa: E731
            dram_nm = _p(name="dram_nm", bufs=2, space="DRAM")
            dram_rs = _p(name="dram_rs", bufs=2, space="DRAM")
            dram_cc = _p(name="dram_cc", bufs=2, space="DRAM")
            singles = _p(name="singles", bufs=1)
            hT_pool = _p(name="hT", bufs=2)
            z_pool = _p(name="zb", bufs=1)
            g_pool = _p(name="gath", bufs=3)
            s_pool = _p(name="sel", bufs=2)
            stg_pool = _p(name="stg", bufs=3)
            agg_pool = _p(name="agg", bufs=3)
            t16_pool = _p(name="t16", bufs=1)
            bn_pool = _p(name="bns", bufs=2)
            stat_pool = _p(name="stat", bufs=2)
            small_pool = _p(name="smal", bufs=2)
            psA = _p(name="psA", bufs=3, space="PSUM")
            psZ = _p(name="psZ", bufs=2, space="PSUM")
            psT = _p(name="psT", bufs=2, space="PSUM")
            ps4_pool = _p(name="ps4", bufs=1, space="PSUM")
            with tc.high_priority():
                nc.gpsimd.load_library(library_config.mlp)
            reg_idx = {th: nc.gpsimd.to_reg(TH_NIDX[th])
                       for th in range(THIRDS)}

            # --- constants / weights in SBUF ---
            # Only dstv/iota (one-hot inputs) load up front; everything else
            # is deferred until after layer-0's partials are emitted so the
            # layer-0 feature loads lead the DMA queue.
            idx_sb = singles.tile([P, IDX_COLS_TOTAL], I16)
            dstv_sb = singles.tile([P, TILES], F16)
            nc.sync.dma_start(out=dstv_sb[:], in_=p_dstv[:])
            iota_sb = singles.tile([P, P], F16)
            nc.sync.dma_start(out=iota_sb[:], in_=p_iota[:])
            ident_sb = singles.tile([P, P], F16)
            wrel_sb = singles.tile([P, L * D], F16)
            wroot_sb = singles.tile([P, L * D], F16)
            w2_sb = singles.tile([P, 2 * OUT], F16)
            bR_sb = singles.tile([1, L * D], F16)
            b2_sb = singles.tile([1, OUT], F16)
            ones_sb = singles.tile([1, P], F16)
            nc.vector.memset(ones_sb[:], 1.0)
            gammaT_sb = singles.tile([P, L], F32)
            betaT_sb = singles.tile([P, L], F32)
            eps_sb = singles.tile([P, 1], F32)
            nc.vector.memset(eps_sb[:], EPS)
            hT_prev = hT_pool.tile([P, SLOTS], F16, tag="hT")

            def emit_late_consts():
                nc.sync.dma_start(out=idx_sb[:], in_=p_idx[:])
                nc.sync.dma_start(out=ident_sb[:], in_=p_ident[:])
                for l in range(L):
                    nc.sync.dma_start(out=wrel_sb[:, l * D : (l + 1) * D],
                                      in_=p_wrel[l])
                    nc.sync.dma_start(out=wroot_sb[:, l * D : (l + 1) * D],
                                      in_=p_wroot[l])
                nc.sync.dma_start(out=w2_sb[:], in_=p_w2[:])
                nc.sync.dma_start(out=bR_sb[:], in_=p_bR[:])
                nc.sync.dma_start(out=b2_sb[:], in_=p_b2[:])
                nc.sync.dma_start(out=gammaT_sb[:], in_=p_gammaT[:])
                nc.sync.dma_start(out=betaT_sb[:], in_=p_betaT[:])
                nc.sync.dma_start(out=hT_prev[:], in_=p_xT[:])

            nm_table = None  # gather source for layers 1..3 (layer 0 uses p_xg)

            def emit_group(th, rd, is_l0, nm_table):
                nidx = TH_NIDX[th]
                tiles = TH_TILES[th]
                t0 = CALL_TILE_OFF[(th, rd)]
                gbuf = g_pool.tile([P, TH_NIDX[0]], F16, tag="gath",
                                   name="gbuf")[:, 0:nidx]
                if is_l0:
                    nc.sync.dma_start(
                        out=gbuf[:],
                        in_=p_xg[:, t0 * D : (t0 + tiles) * D],
                    )
                else:
                    i0 = CALL_ICOL_OFF[(th, rd)]
                    gb3 = gbuf.rearrange("p (t d) -> p t d", t=tiles)
                    nc.gpsimd.dma_gather(
                        out_ap=gb3,
                        in_ap=nm_table[0:SLOTS, :],
                        idxs_ap=idx_sb[:, i0 : i0 + TH_ICOLS[th]],
                        num_idxs=nidx,
                        num_idxs_reg=reg_idx[th],
                        elem_size=D,
                        single_packet=False,
                    )
                sel = s_pool.tile([P, TH_NIDX[0]], F16, tag="sel",
                                  name="sel")[:, 0:nidx]
                dsl = slice(t0, t0 + tiles)
                dstv_bc = bass.AP(
                    tensor=dstv_sb.tensor,
                    offset=dstv_sb[:, dsl].offset,
                    ap=list(dstv_sb[:, dsl].ap) + [[0, P]],
                )
                iota_bc = bass.AP(
                    tensor=iota_sb.tensor,
                    offset=iota_sb[:].offset,
                    ap=[iota_sb[:].ap[0], [0, tiles], iota_sb[:].ap[1]],
                )
                nc.vector.tensor_tensor(
                    out=sel.rearrange("p (t w) -> p t w", t=tiles),
                    in0=dstv_bc,
                    in1=iota_bc,
                    op=mybir.AluOpType.is_equal,
                )
                return gbuf, sel

            def emit_partials_third(th, is_l0, is_final, nm_table, rs_in_t,
                                    rs4_in):
                """One third (TH_CH[th] chunks x 8 dst cores): gather +
                one-hot segment-sum, staged fp16 into rs_in_t (or projected
                to OUT and staged fp32 into rs4_in for the final layer)."""
                tch = TH_CH[th]
                for rd in range(N_CORES):
                    gbuf, sel = emit_group(th, rd, is_l0, nm_table)
                    stage = stg_pool.tile([P, TH_CH[0] * P], F16, tag="stg",
                                          name="stage")[:, 0 : tch * P]
                    ps4 = (ps4_pool.tile([P, TH_CH[0] * OUT], F32, tag="ps4",
                                         name="ps4")[:, 0 : tch * OUT]
                           if is_final else None)
                    bank = None
                    for cl in range(tch):
                        if cl % 4 == 0:
                            bank = psA.tile([P, 512], F32, space="PSUM",
                                            tag="bank")
                        bs = (cl % 4) * P
                        for t in range(TPB):
                            ts = (cl * TPB + t) * P
                            nc.tensor.matmul(
                                out=bank[:, bs : bs + P],
                                lhsT=gbuf[:, ts : ts + P],
                                rhs=sel[:, ts : ts + P],
                                start=(t == 0),
                                stop=(t == TPB - 1),
                            )
                        if cl % 4 == 3 or cl == tch - 1:
                            c0 = (cl // 4) * 4
                            w = (cl - c0 + 1) * P
                            nc.scalar.activation(
                                out=stage[:, c0 * P : c0 * P + w],
                                in_=bank[:, 0:w],
                                func=mybir.ActivationFunctionType.Copy,
                            )
                            if is_final:
                                for c2 in range(c0, cl + 1):
                                    nc.tensor.matmul(
                                        out=ps4[:, c2 * OUT : (c2 + 1) * OUT],
                                        lhsT=stage[:, c2 * P : (c2 + 1) * P],
                                        rhs=w2_sb[:, 0:OUT],
                                        start=True,
                                        stop=True,
                                    )
                    if is_final:
                        z4s = small_pool.tile([P, TH_CH[0] * OUT], F32,
                                              tag="z4s",
                                              name="z4s")[:, 0 : tch * OUT]
                        nc.scalar.activation(
                            out=z4s[:], in_=ps4[:],
                            func=mybir.ActivationFunctionType.Copy,
                        )
                        o0 = TH_OFF[th] * OUT
                        nc.sync.dma_start(
                            out=rs4_in[rd * P : (rd + 1) * P,
                                       o0 : o0 + tch * OUT],
                            in_=z4s[:],
                        )
                    else:
                        nc.sync.dma_start(
                            out=rs_in_t[rd * P : (rd + 1) * P, :], in_=stage[:]
                        )

            def emit_rs_third(th, rs_in_t):
                rs_out_t = dram_rs.tile([P, TH_CH[th] * P], F16,
                                        tag=f"rsout{th}", name="rsout")
                nc.gpsimd.collective_compute(
                    "ReduceScatter", mybir.AluOpType.add, replica_groups=rg,
                    ins=[rs_in_t[:].opt()], outs=[rs_out_t[:]],
                )
                return rs_out_t

            def load_aggT(th, rs_out_t, eng=None):
                aggT_t = agg_pool.tile([P, TH_CH[0] * P], F16, tag="agg",
                                       name="aggT")[:, 0 : TH_CH[th] * P]
                (eng or nc.gpsimd).dma_start(out=aggT_t[:], in_=rs_out_t[:])
                return aggT_t

            def emit_dense_third(l, th, aggT_t, z_all, stats):
                w_rel = wrel_sb[:, l * D : (l + 1) * D]
                w_root = wroot_sb[:, l * D : (l + 1) * D]
                tch = TH_CH[th]
                bank = None
                for cl in range(tch):
                    c = TH_OFF[th] + cl
                    if cl % 4 == 0:
                        bank = psZ.tile([P, 512], F32, space="PSUM", tag="psz",
                                        name="zbank")
                    bs = slice((cl % 4) * P, (cl % 4 + 1) * P)
                    nc.tensor.matmul(
                        out=bank[:, bs], lhsT=w_rel,
                        rhs=aggT_t[:, cl * P : (cl + 1) * P],
                        start=True, stop=False,
                    )
                    nc.tensor.matmul(
                        out=bank[:, bs], lhsT=w_root,
                        rhs=hT_prev[:, c * P : (c + 1) * P],
                        start=False, stop=False,
                    )
                    nc.tensor.matmul(
                        out=bank[:, bs], lhsT=bR_sb[:, l * D : (l + 1) * D],
                        rhs=ones_sb[:], start=False, stop=True,
                    )
                    if cl % 4 == 3 or cl == tch - 1:
                        c0 = (cl // 4) * 4
                        w = (cl - c0 + 1) * P
                        z0 = (TH_OFF[th] + c0) * P
                        nc.scalar.activation(
                            out=z_all[:, z0 : z0 + w], in_=bank[:, 0:w],
                            func=mybir.ActivationFunctionType.Copy,
                        )
                        for c2 in range(c0, cl + 1):
                            zc = (TH_OFF[th] + c2) * P
                            nc.vector.bn_stats(
                                out=stats[:, TH_OFF[th] + c2, :],
                                in_=z_all[:, zc : zc + P],
                            )

            def emit_apply(z_all, scale, shift):
                hT_new = hT_pool.tile([P, SLOTS], F16, tag="hT")
                t16_all = t16_pool.tile([P, CHUNKS, P], F16, tag="t16")
                nm_new = dram_nm.tile([SLOTS, D], F16, tag="nm")
                for th in (THIRDS - 1, *range(THIRDS - 1)):
                    o = TH_OFF[th]
                    n = TH_CH[th]
                    tsl = slice(o * P, (o + n) * P)
                    nc.scalar.activation(
                        out=hT_new[:, tsl], in_=z_all[:, tsl],
                        func=mybir.ActivationFunctionType.Relu,
                        bias=shift, scale=scale,
                    )
                    bank = None
                    for cl in range(n):
                        c = o + cl
                        cs = slice(c * P, (c + 1) * P)
                        if cl % 4 == 0:
                            bank = psT.tile([P, 512], F16, space="PSUM",
                                            tag="pst")
                        nc.tensor.transpose(
                            out=bank[:, (cl % 4) * P : (cl % 4 + 1) * P],
                            in_=hT_new[:, cs], identity=ident_sb[:],
                        )
                        if cl % 4 == 3 or cl == n - 1:
                            c0 = (cl // 4) * 4
                            w = (cl - c0 + 1) * P
                            nc.vector.tensor_copy(
                                out=t16_all.rearrange("p c f -> p (c f)")[
                                    :, (o + c0) * P : (o + c0) * P + w
                                ],
                                in_=bank[:, 0:w],
                            )
                    # per-third slice of the node-major table: rows w*51+c,
                    # c in [o, o+n) -> per partition a contiguous n*256B run
                    nm_rows = bass.AP(
                        tensor=nm_new.tensor,
                        offset=nm_new[:].offset + o * D,
                        ap=[[CHUNKS * D, P], [D, n], [1, D]],
                    )
                    t16_src = bass.AP(
                        tensor=t16_all.tensor,
                        offset=t16_all[:].offset + o * P,
                        ap=[t16_all[:].ap[0], [P, n], [1, P]],
                    )
                    nc.sync.dma_start(out=nm_rows, in_=t16_src)
                return nm_new, hT_new

            def emit_final_root():
                """Root term of the last GraphConv — depends only on the
                layer-3 input features, so it runs under the gather phase."""
                psr_full = psZ.tile([P, 512], F32, space="PSUM", tag="psz",
                                    name="psr")
                root_sb = small_pool.tile([P, CHUNKS * OUT], F32, tag="root")
                for c in range(CHUNKS):
                    cs = slice(c * P, (c + 1) * P)
                    psr = psr_full[:, c * OUT : (c + 1) * OUT]
                    nc.tensor.matmul(
                        out=psr, lhsT=hT_prev[:, cs],
                        rhs=w2_sb[:, OUT : 2 * OUT],
                        start=True, stop=False,
                    )
                    nc.tensor.matmul(
                        out=psr, lhsT=ones_sb[:], rhs=b2_sb[:],
                        start=False, stop=True,
                    )
                nc.scalar.activation(
                    out=root_sb[:], in_=psr_full[:, 0 : CHUNKS * OUT],
                    func=mybir.ActivationFunctionType.Copy,
                )
                return root_sb

            def emit_final(rs4_out, root_sb):
                rs4_sb = small_pool.tile([P, CHUNKS * OUT], F32, tag="rs4sb")
                nc.sync.dma_start(out=rs4_sb[:], in_=rs4_out[:])
                out_sb = small_pool.tile([P, CHUNKS * OUT], F32, tag="outsb")
                nc.vector.tensor_tensor(
                    out=out_sb[:], in0=root_sb[:], in1=rs4_sb[:],
                    op=mybir.AluOpType.add,
                )
                nc.sync.dma_start(out=p_out[:], in_=out_sb[:])

            for l in range(L + 1):
                is_final = l == L
                is_l0 = l == 0

                if is_final:
                    rs4_in = dram_rs.tile([N_CORES * P, CHUNKS * OUT], F32,
                                          tag="rsin4")
                    root_sb = emit_final_root()
                    for th in range(THIRDS):
                        emit_partials_third(th, is_l0, True, nm_table, None,
                                            rs4_in)
                    rs4_out = dram_rs.tile([P, CHUNKS * OUT], F32, tag="rsout4")
                    nc.gpsimd.collective_compute(
                        "ReduceScatter", mybir.AluOpType.add,
                        replica_groups=rg,
                        ins=[rs4_in[:].opt()], outs=[rs4_out[:]],
                    )
                    emit_final(rs4_out, root_sb)
                    continue

                z_all = z_pool.tile([P, SLOTS], F32, tag="z")
                stats = stat_pool.tile([P, CHUNKS, nc.vector.BN_STATS_DIM],
                                       F32, tag="stats")
                # issue order: ALL partials before ANY dense, so no engine
                # queue entry of the gather/agg pipeline sits behind a
                # ReduceScatter-gated dense instruction. aggT(th-2) preloads
                # on the Act queue between thirds (its slot is idle there).
                rs_outs = {}
                aggTs = {}
                for th in range(THIRDS):
                    if th >= 2:
                        aggTs[th - 2] = load_aggT(th - 2, rs_outs[th - 2],
                                                  eng=nc.scalar)
                    rs_in_t = dram_rs.tile([N_CORES * P, TH_CH[th] * P], F16,
                                           tag=f"rsin{th}", name="rsin")
                    emit_partials_third(th, is_l0, False, nm_table, rs_in_t,
                                        None)
                    rs_outs[th] = emit_rs_third(th, rs_in_t)
                if is_l0:
                    emit_late_consts()
                for th in range(THIRDS):
                    if th not in aggTs:
                        aggTs[th] = load_aggT(
                            th, rs_outs[th],
                            eng=nc.scalar if th == THIRDS - 1 else None,
                        )
                    emit_dense_third(l, th, aggTs[th], z_all, stats)

                # ---- BatchNorm: global stats with pad-slot correction ----
                bs = bn_pool.tile([P, 16], F32, tag="bn")
                mv = bs[:, 0:2]
                with tc.high_priority():
                    nc.vector.bn_aggr(out=mv, in_=stats[:])
                # M2_all = mean^2 + var; pad-corrected sums, pre-divided by
                # N_CORES so the post-collective reduce directly yields the
                # global mean/E2: Mk = (Mk_all*(S/NPC) - zp^k*(PAD/NPC))/8
                c1 = SLOTS / NODES_PER_CORE / N_CORES
                c2 = N_PADS / NODES_PER_CORE / N_CORES
                zp = z_all[:, SLOTS - 1 : SLOTS]
                cc_sb = bs[:, 3:5]
                with tc.high_priority():
                    # cc0 = mean_all*c1 - zp*c2
                    nc.vector.tensor_scalar(
                        out=bs[:, 5:6], in0=zp, scalar1=c2, scalar2=None,
                        op0=mybir.AluOpType.mult,
                    )
                    nc.vector.tensor_scalar(
                        out=cc_sb[:, 0:1], in0=mv[:, 0:1], scalar1=c1,
                        scalar2=bs[:, 5:6], op0=mybir.AluOpType.mult,
                        op1=mybir.AluOpType.subtract,
                    )
                    # cc1 = (mean^2+var)*c1 - zp^2*c2
                    nc.vector.tensor_scalar(
                        out=bs[:, 6:7], in0=mv[:, 0:1], scalar1=mv[:, 0:1],
                        scalar2=mv[:, 1:2], op0=mybir.AluOpType.mult,
                        op1=mybir.AluOpType.add,
                    )
                    nc.vector.tensor_tensor(
                        out=bs[:, 7:8], in0=zp, in1=zp, op=mybir.AluOpType.mult,
                    )
                    nc.vector.tensor_scalar(
                        out=bs[:, 8:9], in0=bs[:, 7:8], scalar1=c2, scalar2=None,
                        op0=mybir.AluOpType.mult,
                    )
                    nc.vector.tensor_scalar(
                        out=cc_sb[:, 1:2], in0=bs[:, 6:7], scalar1=c1,
                        scalar2=bs[:, 8:9], op0=mybir.AluOpType.mult,
                        op1=mybir.AluOpType.subtract,
                    )
                cc_in = dram_cc.tile([P, 2], F32, tag="ccin")
                cc_out = dram_cc.tile([P * N_CORES, 2], F32, addr_space="Shared",
                                      tag="ccout")
                nc.sync.dma_start(out=cc_in[:], in_=cc_sb)
                nc.gpsimd.collective_compute(
                    "AllGather", mybir.AluOpType.bypass, replica_groups=rg,
                    ins=[cc_in.opt()], outs=[cc_out.opt()],
                )
                cc_all = bn_pool.tile([P, 2, N_CORES], F32, tag="ccall")
                cc_src = bass.AP(
                    tensor=cc_out.tensor,
                    offset=cc_out[:].offset,
                    ap=[[2, P], [1, 2], [2 * P, N_CORES]],
                )
                nc.sync.dma_start(out=cc_all[:], in_=cc_src)
                cc_res = bs[:, 9:11]
                nc.vector.tensor_reduce(
                    out=cc_res.rearrange("p (a b) -> p a b", a=2),
                    in_=cc_all[:],
                    axis=mybir.AxisListType.X,
                    op=mybir.AluOpType.add,
                )
                mu = cc_res[:, 0:1]
                # var = E2 - mu^2 in one fused op
                var = bs[:, 12:13]
                nc.vector.tensor_scalar(
                    out=bs[:, 13:14], in0=mu, scalar1=mu, scalar2=None,
                    op0=mybir.AluOpType.mult,
                )
                nc.vector.tensor_tensor(
                    out=var, in0=cc_res[:, 1:2], in1=bs[:, 13:14],
                    op=mybir.AluOpType.subtract,
                )
                rstd = bs[:, 14:15]
                nc.scalar.activation(
                    out=rstd, in_=var,
                    func=mybir.ActivationFunctionType.Sqrt,
                    bias=eps_sb[:], scale=1.0,
                )
                nc.vector.reciprocal(out=rstd, in_=rstd)
                scale = bs[:, 15:16]
                nc.vector.tensor_tensor(
                    out=scale, in0=rstd, in1=gammaT_sb[:, l : l + 1],
                    op=mybir.AluOpType.mult,
                )
                shift = bs[:, 2:3]
                nc.vector.tensor_tensor(
                    out=shift, in0=mu, in1=scale, op=mybir.AluOpType.mult
                )
                nc.vector.tensor_tensor(
                    out=shift, in0=betaT_sb[:, l : l + 1], in1=shift,
                    op=mybir.AluOpType.subtract,
                )

                # ---- BN apply + ReLU, transpose to node-major fp16 table ----
                nm_table, hT_prev = emit_apply(z_all, scale, shift)

    lower_extended_insts(nc)
    _split_multiwait(nc)
    return nc


_PROGRAM = None


def _get_program():
    global _PROGRAM
    if _PROGRAM is None:
        _PROGRAM = build_program()
    return _PROGRAM


def run(x, edge_index, Wrel, Wroot, b, gamma, beta, Wrel2, Wroot2, b2):
    """Returns (output [N, OUT] float32, nc) — nc exposed for profiling."""
    core_of, chunk_of, slot_of, idx_cores, dstv_cores, x_nm, xT, xg = _preprocess(
        x, edge_index
    )
    nc = _get_program()

    iota16 = np.broadcast_to(np.arange(P, dtype=np.float16), (P, P)).copy()
    ident16 = np.eye(P, dtype=np.float16)
    w2 = np.concatenate(
        [np.asarray(Wrel2, np.float32), np.asarray(Wroot2, np.float32)], axis=1
    ).astype(np.float16)
    common = dict(
        wrel=np.ascontiguousarray(np.asarray(Wrel, np.float32)).astype(np.float16),
        wroot=np.ascontiguousarray(np.asarray(Wroot, np.float32)).astype(np.float16),
        w2=np.ascontiguousarray(w2),
        bR=np.asarray(b, np.float32).reshape(1, L * D).astype(np.float16),
        b2=np.asarray(b2, np.float32).reshape(1, OUT).astype(np.float16),
        gammaT=np.ascontiguousarray(np.asarray(gamma, np.float32).T),
        betaT=np.ascontiguousarray(np.asarray(beta, np.float32).T),
        iota16=iota16,
        ident16=ident16,
    )
    in_maps = []
    for c in range(N_CORES):
        m = dict(common)
        m["xg"] = xg[c]
        m["xT_loc"] = xT[c]
        m["gidx"] = idx_cores[c]
        m["dstv"] = dstv_cores[c]
        in_maps.append(m)

    from concourse.bass_utils import run_bass_kernel_spmd

    res = run_bass_kernel_spmd(nc, in_maps, list(range(N_CORES)))
    # z4 per core: [P(slot), CHUNKS*OUT] -> full[node]
    full = np.empty((N, OUT), np.float32)
    for r in range(N_CORES):
        z4 = res.results[r]["z4"].reshape(P, CHUNKS, OUT)
        m = core_of == r
        full[m] = z4[slot_of[m], chunk_of[m], :]
    return full, nc


def kernel(**inputs):
    out, _ = run(**{k: np.asarray(v) for k, v in inputs.items()})
    return out
